# revision 29
# baseline (speedup 1.0000x reference)
"""Trainium2 Bass kernel for nn_Encoder_LSTM (4x LSTMCell with zero state over
packed ragged tokens).

Math (from the reference): all rows independent; for each output row j with
source row s(j) (the ragged gather), and each of 4 layers:
    gates = x @ W_ih^T + (b_ih + b_hh);  i, f, g, o = split(gates)
    c = sigmoid(i) * tanh(g);  h = sigmoid(o) * tanh(c)      (f is unused)
Outputs: (output=h4, h1, c1, h2, c2, h3, c3, h4, c4), each [sum(bs), 512] fp32.

Strategy (feature-major / weights-stationary, PE-roofline-bound):
  - Only U = max_j s(j)+1 source rows are distinct; compute each distinct row
    once on-device, replicate to duplicate output rows on the host during
    unshard.  Shard distinct rows round-robin (r mod 8 -> core; 2056/core).
  - Tokens live on the matmul FREE axis: gates^T = W @ x^T come out of PE as
    [gate_chunk(128), tok] tiles, per-gate bias fused into the ACT
    evacuation, h^T feeds the next layer's matmul directly.  PE runs at its
    16-bit roofline (~165 us of pure matmul per core = the hard floor; fp8
    was measured 10x outside the accuracy budget).
  - fp16 (not bf16) on the wire and in the matmuls: same PE/DVE/DMA cost,
    8x lower quantization error (1.8e-3 vs 1.4e-2 rel absmax).
  - PE warm-up: a short stream of dummy matmuls on memset tiles runs during
    the input-DMA wait so the HAM clock-gate reaches 2.4 GHz before the real
    matmul stream begins (saves ~2.5 us of cold-rate matmul).
  - DMA descriptor diet: weights packed [l][gc][kc][m] so each weight DMA is
    one contiguous per-partition run; x packed group-major ([128, 4*ntok]
    contiguous per group); h+c shipped as one contiguous [128, 8*ntok] block
    per (layer, group).  ~6k descriptors instead of ~31k; startup desc-gen
    runs in parallel on both HWDGE rings (sync: weights, scalar: x).
  - Tail: the last (smallest, 264-token) group pipelines sigmoid(o) -> h ->
    DMA per 128-feature chunk, with tanh(c) hoisted before the o-gate
    evacuations, so the post-last-matmul chain is ~1.5 us.
  - Host only: input gather+transpose, weight packing, output reassembly.
"""

import sys

if "/opt/trn_rl_repo" not in sys.path:
    sys.path.insert(0, "/opt/trn_rl_repo")

import numpy as np

P = 128
H = 512
TOK = 512          # max tokens per group (PSUM tile [128, TOK] f32 = one bank)
PSN = 512          # PSUM tile free size (one bank)
NCORES = 8
NL = 4             # layers
NGC = 12           # gate chunks per layer: 0-3 = i, 4-7 = g, 8-11 = o
NWARM = 11         # PE warm-up matmuls (N=416; ~5.3 us bridging to 1st real MM)
WARMN = 416        # warm-up matmul free dim


# ---------------------------------------------------------------- host plan

def _make_plan(batch_sizes):
    bs = np.asarray(batch_sizes).astype(np.int64)
    s = np.concatenate([i * b + np.arange(b) for i, b in enumerate(bs)]).astype(np.int64)
    Nout = int(s.size)
    U = int(s.max()) + 1
    n_per_core = [len(range(c, U, NCORES)) for c in range(NCORES)]
    NT = ((max(n_per_core) + 1) // 2) * 2       # even; no other padding needed
    # Group sizing: every matmul N >= 257 keeps the 107ns LDWEIGHTS hidden
    # behind the matmul stream, and the last group must stay >= ~372 so it
    # is not ACT-bound (16 ACT insts x ~200ns fixed cost vs 48 matmuls),
    # which would stretch the end-of-kernel tail.
    # every matmul N a multiple of 8, <= 512 (one PSUM bank; the device
    # compiler rejects larger), last group smallest but ACT-balanced
    G = -(-NT // TOK)
    size8 = -(-(NT // G) // 8) * 8
    sizes = []
    left = NT
    while left > 0:
        sizes.append(min(size8, left))
        left -= sizes[-1]
    sizes.sort(reverse=True)
    groups = []
    g0 = 0
    for ntok in sizes:
        groups.append((g0, ntok))
        g0 += ntok
    return dict(s=s, Nout=Nout, U=U, NT=NT, groups=groups)


def _pack_weights(inputs):
    """-> w_host [128, NL*NGC*4*128] f16 (lhsT blocks, [l][gc][kc][m] order so
    per-(l, gc-range) DMA slices are contiguous), b_host [128, NL*NGC] f32."""
    w = np.zeros((NL, NGC, 4, P, P), np.float32)   # [l, gc, kc, feat_p, gate_m]
    b = np.zeros((P, NL * NGC), np.float32)
    for li in range(NL):
        W = np.asarray(inputs[f"W_ih{li+1}"], np.float32)          # [4H, 512]
        bb = (np.asarray(inputs[f"b_ih{li+1}"], np.float32)
              + np.asarray(inputs[f"b_hh{li+1}"], np.float32))     # [4H]
        # gate order i, f, g, o; f unused. pack [i, g, o].
        Wigo = np.concatenate([W[0:H], W[2 * H:3 * H], W[3 * H:4 * H]], axis=0)
        bigo = np.concatenate([bb[0:H], bb[2 * H:3 * H], bb[3 * H:4 * H]])
        for gc in range(NGC):
            for kc in range(4):
                w[li, gc, kc] = Wigo[gc * P:(gc + 1) * P, kc * P:(kc + 1) * P].T
        b[:, li * NGC:(li + 1) * NGC] = bigo.reshape(NGC, P).T
    w_host = np.ascontiguousarray(
        w.transpose(3, 0, 1, 2, 4).reshape(P, -1)).astype(np.float16)
    return w_host, b


def _pack_x(xt, NT, groups):
    """xt [n, 512] f16 -> [128, 4*NT] group-major: group g occupies columns
    [4*g0, 4*g0+4*ntok), chunk c of the contraction at +c*ntok."""
    n = xt.shape[0]
    xc = np.zeros((H, NT), np.float16)
    xc[:, :n] = xt.T
    out = np.empty((P, 4 * NT), np.float16)
    for g0, ntok in groups:
        blk = xc[:, g0:g0 + ntok].reshape(4, P, ntok).transpose(1, 0, 2)
        out[:, 4 * g0:4 * g0 + 4 * ntok] = blk.reshape(P, 4 * ntok)
    return out


# ---------------------------------------------------------------- bass build

def _build_nc(NT, groups):
    import concourse.mybir as mybir
    from concourse import bacc
    from concourse.tile import TileContext

    dt = mybir.dt
    AF = mybir.ActivationFunctionType

    WCOLS = NL * NGC * 4 * P            # 24576

    nc = bacc.Bacc()
    x_d = nc.dram_tensor("x", [P, 4 * NT], dt.float16, kind="ExternalInput")
    w_d = nc.dram_tensor("w", [P, WCOLS], dt.float16, kind="ExternalInput")
    b_d = nc.dram_tensor("b", [P, NL * NGC], dt.float32, kind="ExternalInput")
    # hc[l, p, col]: per (layer, group) one contiguous block at col 8*g0:
    # c chunks at +c*ntok, h chunks at +4*ntok+c*ntok.
    o_d = nc.dram_tensor("hc", [NL, P, 8 * NT], dt.float16,
                         kind="ExternalOutput")

    def wcol(li, kc, gc):
        return (((li * NGC) + gc) * 4 + kc) * P

    with TileContext(nc) as tc:
        with (
            tc.tile_pool(name="const", bufs=1) as constp,
            tc.tile_pool(name="ew", bufs=3) as ewp,
            tc.tile_pool(name="hc", bufs=7) as hcp,
            tc.tile_pool(name="ps", bufs=8, space="PSUM") as psp,
        ):
            w_sb = constp.tile([P, WCOLS], dt.float16)
            x_sb = constp.tile([P, 4 * NT], dt.float16)
            b_sb = constp.tile([P, NL * NGC], dt.float32)

            # Prewarm the ACT spline tables (sigmoid + tanh) FIRST: the table
            # loads are DMAs that stall the scalar DGE ring for ~2.6us, so
            # running them before the x wire traffic keeps the startup
            # deterministic.  Then stream dummy matmuls on memset tiles so
            # the PE HAM clock-gate reaches (and holds) 2.4 GHz until the
            # first real matmul's data arrives -- sized to overshoot
            # slightly; overrun costs warm-rate dummies, undershoot would
            # re-throttle the whole stream to 1.2 GHz for ~4us.
            warm = constp.tile([P, 2], dt.float32)
            wx = constp.tile([P, WARMN], dt.float16)
            ww = constp.tile([P, P], dt.float16)
            nc.vector.memset(warm[:], 0.0)
            nc.vector.memset(wx[:], 0.0)
            nc.vector.memset(ww[:], 0.0)
            nc.scalar.activation(warm[:, 0:1], warm[:, 1:2], AF.Sigmoid)
            nc.scalar.activation(warm[:, 0:1], warm[:, 1:2], AF.Tanh)
            ps_w = psp.tile([P, PSN], dt.float32, tag="ps")
            for _ in range(NWARM):
                nc.tensor.matmul(ps_w[:, :WARMN], ww[:], wx[:],
                                 start=True, stop=True)

            # startup: both HWDGE rings desc-gen in parallel.  sync ring:
            # weights (first the single block the first matmuls need);
            # scalar ring: x group 0 chunk-by-chunk (the first matmul only
            # has to wait for chunk kc=0), then the remaining groups.
            nc.sync.dma_start(w_sb[:, wcol(0, 0, 4):wcol(0, 0, 5)],
                              w_d[:, wcol(0, 0, 4):wcol(0, 0, 5)])
            g00, ntok0 = groups[0]
            nc.scalar.dma_start(x_sb[:, 4 * g00:4 * g00 + 4 * ntok0],
                                x_d[:, 4 * g00:4 * g00 + 4 * ntok0])
            nc.sync.dma_start(w_sb[:, wcol(0, 0, 5):wcol(0, 0, 8)],
                              w_d[:, wcol(0, 0, 5):wcol(0, 0, 8)])
            nc.sync.dma_start(w_sb[:, wcol(0, 0, 0):wcol(0, 0, 4)],
                              w_d[:, wcol(0, 0, 0):wcol(0, 0, 4)])
            nc.sync.dma_start(w_sb[:, wcol(0, 0, 8):wcol(0, 0, 12)],
                              w_d[:, wcol(0, 0, 8):wcol(0, 0, 12)])
            nc.sync.dma_start(b_sb[:], b_d[:])

            # Remaining loads are deferred into the emission stream (below)
            # so the startup wire carries only what the first group needs:
            # an oversubscribed wire at startup makes x/w arrival jittery
            # across cores and any late group stalls the matmul stream.
            def load_x_group(g0, ntok):
                nc.scalar.dma_start(x_sb[:, 4 * g0:4 * g0 + 4 * ntok],
                                    x_d[:, 4 * g0:4 * g0 + 4 * ntok])

            def load_w_layer(li):
                for gl, gh in ((4, 8), (0, 4), (8, 12)):
                    nc.sync.dma_start(w_sb[:, wcol(li, 0, gl):wcol(li, 0, gh)],
                                      w_d[:, wcol(li, 0, gl):wcol(li, 0, gh)])

            def bias(li, gc):
                j = li * NGC + gc
                return b_sb[:, j:j + 1]

            def emit_layer(li, g0, ntok, rhs_of, split_tail=False):
                """rhs_of(c) -> [P, ntok] f16 AP (chunk c of this layer's
                input, feature-major).  Returns the hc tile; h lives at
                [:, 4*ntok + c*ntok] for chunk c.

                ACT (the only engine that can do sigmoid/tanh) is the ~162us
                bottleneck when it both evacuates PSUM and applies the
                per-gate-chunk bias: 12 narrow instructions per group at
                ~200ns fixed cost each.  Instead DVE evacuates each PSUM bank
                with a fused per-partition bias add (tensor_scalar_add, which
                also frees the bank for the matmul stream ~1.5x faster), and
                ACT runs ONE wide activation per gate: 4 ACT instructions
                per group, ~126us total."""
                W4 = 4 * ntok

                def mm(gc):
                    ps = psp.tile([P, PSN], dt.float32, tag="ps")
                    for kc in range(4):
                        nc.tensor.matmul(ps[:, :ntok],
                                         w_sb[:, wcol(li, kc, gc):
                                              wcol(li, kc, gc) + P],
                                         rhs_of(kc),
                                         start=(kc == 0), stop=(kc == 3))
                    return ps

                si = ewp.tile([P, 4 * TOK], dt.float16, tag="si")
                tg = ewp.tile([P, 4 * TOK], dt.float16, tag="tg")
                so = ewp.tile([P, 4 * TOK], dt.float16, tag="so")
                tcl = ewp.tile([P, 4 * TOK], dt.float16, tag="tc")
                hc = hcp.tile([P, 8 * TOK], dt.float16, tag="hc")
                c_t = hc[:, 0:W4]
                h_t = hc[:, W4:2 * W4]

                # g gates first: the tanh(g) -> c -> tanh(c) -> h chain is
                # the critical path into the next layer.
                for gc in range(4):      # g chunks: tanh(psum + bias) on ACT
                    ps = mm(4 + gc)
                    nc.scalar.activation(tg[:, gc * ntok:(gc + 1) * ntok],
                                         ps[:, :ntok], AF.Tanh,
                                         bias=bias(li, 4 + gc))
                for gc in range(4):      # i chunks: sigmoid(psum + bias)
                    ps = mm(gc)
                    sl = slice(gc * ntok, (gc + 1) * ntok)
                    nc.scalar.activation(si[:, sl], ps[:, :ntok], AF.Sigmoid,
                                         bias=bias(li, gc))
                    # per-chunk c mul: hides behind the next chunk's ACT
                    nc.vector.tensor_mul(c_t[:, sl], si[:, sl], tg[:, sl])
                if split_tail:
                    # last step of the kernel: ship c early, hoist tanh(c)
                    # over the o matmuls, then pipeline sigmoid(o) -> h ->
                    # small DMA per 128-feature chunk.
                    nc.sync.dma_start(o_d[li, :, 8 * g0:8 * g0 + W4], c_t)
                    nc.scalar.activation(tcl[:, :W4], c_t, AF.Tanh)
                    for gc in range(4):
                        ps = mm(8 + gc)
                        sl = slice(gc * ntok, (gc + 1) * ntok)
                        nc.scalar.activation(so[:, sl], ps[:, :ntok],
                                             AF.Sigmoid, bias=bias(li, 8 + gc))
                        nc.vector.tensor_mul(h_t[:, sl], so[:, sl], tcl[:, sl])
                        nc.sync.dma_start(
                            o_d[li, :, 8 * g0 + W4 + gc * ntok:
                                8 * g0 + W4 + (gc + 1) * ntok],
                            h_t[:, sl])
                else:
                    for gc in range(4):  # o chunks: sigmoid(psum + bias)
                        ps = mm(8 + gc)
                        nc.scalar.activation(so[:, gc * ntok:(gc + 1) * ntok],
                                             ps[:, :ntok], AF.Sigmoid,
                                             bias=bias(li, 8 + gc))
                    nc.scalar.activation(tcl[:, :W4], c_t, AF.Tanh)
                    nc.vector.tensor_mul(h_t, so[:, :W4], tcl[:, :W4])
                    # one contiguous [128, 8*ntok] block: c then h
                    nc.sync.dma_start(o_d[li, :, 8 * g0:8 * g0 + 2 * W4],
                                      hc[:, :2 * W4])
                return hc

            def emit_group_layer(li, grp, hcprev, split_tail=False):
                g0, ntok = grp
                if li == 0:
                    rhs_of = lambda c: x_sb[:, 4 * g0 + c * ntok:
                                            4 * g0 + (c + 1) * ntok]
                else:
                    rhs_of = lambda c: hcprev[:, (4 + c) * ntok:
                                              (5 + c) * ntok]
                return emit_layer(li, g0, ntok, rhs_of, split_tail=split_tail)

            # interleave all groups per layer so PE never waits on a layer's
            # elementwise tail; deferred input loads are emitted one group /
            # one layer ahead of first use so the wire stays ~2 groups ahead
            # of the matmul stream without ever being oversubscribed.
            hcprevs = [None] * len(groups)
            NG = len(groups)
            for li in range(NL):
                for k, grp in enumerate(groups):
                    if li == 0 and k + 1 < NG:
                        load_x_group(*groups[k + 1])
                    if k == 2 and li + 1 < NL:
                        load_w_layer(li + 1)
                    is_last = (li == NL - 1 and k == NG - 1)
                    hcprevs[k] = emit_group_layer(li, grp, hcprevs[k],
                                                  split_tail=is_last)
    nc.compile()
    return nc


# ---------------------------------------------------------------- entry point

def _ensure_axon_hooks():
    """bass_utils' trace path imports antenv.axon_hooks, which some images
    lack; install a shim that drives NTFF profiling via libaxon_pjrt.so
    (mirrors the boot-side _ntff_profile_via_ctypes) or degrades to None."""
    try:
        import antenv.axon_hooks  # noqa: F401
        return
    except ImportError:
        pass
    import types
    import contextlib
    import ctypes

    def _build_hook():
        so = "/opt/axon/libaxon_pjrt.so"
        try:
            lib = ctypes.CDLL(so)
        except OSError:
            return None
        if not hasattr(lib, "axon_start_nrt_profile"):
            return None
        lib.axon_start_nrt_profile.argtypes = [
            ctypes.POINTER(ctypes.c_int64), ctypes.c_size_t]
        lib.axon_start_nrt_profile.restype = ctypes.c_int64
        lib.axon_stop_nrt_profile.argtypes = [ctypes.c_char_p]
        lib.axon_stop_nrt_profile.restype = ctypes.c_int64

        @contextlib.contextmanager
        def _hook(output_dir, device_ids):
            import jax
            jax.devices()
            if device_ids:
                ids = (ctypes.c_int64 * len(device_ids))(*device_ids)
                rc = lib.axon_start_nrt_profile(ids, len(device_ids))
            else:
                rc = lib.axon_start_nrt_profile(None, 0)
            if rc != 0:
                raise RuntimeError(f"axon_start_nrt_profile rc={rc}")
            try:
                yield
            finally:
                n = lib.axon_stop_nrt_profile(str(output_dir).encode())
                print(f"ntff profile: {n} file(s) written to {output_dir}",
                      file=sys.stderr)

        return _hook

    box = [None, False]

    def set_axon_ntff_profile_hook(h):
        box[0] = h
        box[1] = True

    def get_axon_ntff_profile_hook():
        if not box[1]:
            box[0] = _build_hook()
            box[1] = True
        return box[0]

    mod = types.ModuleType("antenv.axon_hooks")
    mod.set_axon_ntff_profile_hook = set_axon_ntff_profile_hook
    mod.get_axon_ntff_profile_hook = get_axon_ntff_profile_hook
    import antenv
    sys.modules["antenv.axon_hooks"] = mod
    antenv.axon_hooks = mod


_cache = {}


def kernel(**inputs):
    packed_x = np.asarray(inputs["packed_x"], np.float32)
    bs = np.asarray(inputs["batch_sizes"])

    key = bs.tobytes()
    if key not in _cache:
        plan = _make_plan(bs)
        nc = _build_nc(plan["NT"], plan["groups"])
        _cache[key] = (plan, nc)
    plan, nc = _cache[key]

    w, b = _pack_weights(inputs)
    NT, U, s, Nout = plan["NT"], plan["U"], plan["s"], plan["Nout"]
    groups = plan["groups"]

    in_maps = []
    for c in range(NCORES):
        src = np.arange(c, U, NCORES, dtype=np.int64)
        x = _pack_x(packed_x[src].astype(np.float16), NT, groups)
        in_maps.append({"x": x, "w": w, "b": b})

    from concourse.bass_utils import run_bass_kernel_spmd
    _ensure_axon_hooks()
    res = run_bass_kernel_spmd(nc, in_maps, core_ids=list(range(NCORES)))
    global last_result
    last_result = res

    # reassemble: per (core, layer, kind) build the feature-major [512, NT]
    # view from the group-major blocks, then scatter to output rows.
    core_of = (s % NCORES).astype(np.int64)
    pos_of = (s // NCORES).astype(np.int64)
    slabs = [np.asarray(res.results[c]["hc"]).reshape(NL, P, 8 * NT)
             for c in range(NCORES)]
    views = {}   # (core, li, kind) -> [512, NT] f16
    for c in range(NCORES):
        for li in range(NL):
            for kind in (0, 1):      # 0 = c, 1 = h
                V = np.empty((H, NT), np.float16)
                for g0, ntok in groups:
                    koff = 8 * g0 + kind * 4 * ntok
                    blk = slabs[c][li][:, koff:koff + 4 * ntok]
                    V[:, g0:g0 + ntok] = (
                        blk.reshape(P, 4, ntok).transpose(1, 0, 2)
                        .reshape(H, ntok))
                views[(c, li, kind)] = V

    full = {}
    names = ["h1", "c1", "h2", "c2", "h3", "c3", "h4", "c4"]
    for j, nm in enumerate(names):
        li, kind = j // 2, 1 - (j % 2)
        f = np.empty((Nout, H), np.float32)
        for c in range(NCORES):
            js = np.flatnonzero(core_of == c)
            f[js] = views[(c, li, kind)][:, pos_of[js]].T
        full[nm] = f

    return (full["h4"], full["h1"], full["c1"], full["h2"], full["c2"],
            full["h3"], full["c3"], full["h4"], full["c4"])


if __name__ == "__main__":
    import reference
    inputs = reference.setup_inputs()
    out = kernel(**{k: np.asarray(v) for k, v in inputs.items()})
    print([o.shape for o in out])
